# revision 31
# baseline (speedup 1.0000x reference)
"""MACE-style GNN message passing on 8 Trainium2 NeuronCores.

Only the l=0 (scalar) channel of the reference network reaches the output:
Y[:,0] == 1 and the readout consumes out[:, :, 0] alone, so the whole l>=1
spherical-harmonic pipeline is dead code.  What remains per edge is the
radial MLP (8->64->64->64->128), a per-sender-species channel scale, and a
scatter-add over receivers.  Node-side work collapses to per-species table
lookups (implemented as one-hot matmuls) plus three K x K matmuls.

Sharding: receivers are range-partitioned over the 8 cores (1000 nodes
each, padded to 8 tiles of 128).  Each core gets the edges targeting its
nodes, grouped by 128-node tile and padded to a uniform segment length so
all cores run one SPMD program.  Edges with r >= r_max (zero envelope) are
dropped on the host.

All matmuls keep operands on full 128-partition tiles at base partition 0
(tile_position (0,0)); sub-128 contractions are realized with zero-padded
block weights.  Partial-partition operands on compute-written tiles
misread on this hardware.
"""

import sys
import numpy as np

sys.path.insert(0, "/opt/trn_rl_repo")

R_MAX = 5.0
EPS = 1e-9
AVG = 16.0
N_NODES = 8000
Z = 10
K = 128
NB = 8
NCORES = 8
NPC = N_NODES // NCORES       # nodes per core
NT = 8                        # node tiles per core (128 nodes each)
NPAD = NT * 128               # padded nodes per core
ET_EDGES = 512                # edges per device tile (4 x 128 subtiles)

_CONST_SPECS = [
    ("iota", 128), ("cb4n", 32),
    ("onei", 1), ("neg1i", 1), ("magic", 1),
]
CONST_W = sum(w for _, w in _CONST_SPECS)

# fp16 constants (matmul weights; PE runs fp16 at 1 cycle/row vs 4 for fp32)
_CONSTH_SPECS = [
    ("i128", 128),
    ("w1a", 128), ("w1b", 128),
    ("w2", 128), ("w3", 128),
    ("w40a", 128), ("w40b", 128),
    ("wlin0", 128), ("wsym0", 128),
    ("wlin20", 128), ("sct", 128), ("ae", 1), ("wro", 1),
]
CONSTH_W = sum(w for _, w in _CONSTH_SPECS)


def _spec_cols(specs):
    cols, off = {}, 0
    for name, w in specs:
        cols[name] = (off, off + w)
        off += w
    return cols


TRACE = False
LAST_RESULTS = None

_prog_cache = {}


def _build_program(ET):
    """Build the SPMD Bass program for ET 512-edge tiles per node tile."""
    from concourse import bass, bacc, mybir
    from concourse.tile import TileContext

    f32 = mybir.dt.float32
    f16 = mybir.dt.float16
    i32 = mybir.dt.int32
    AF = mybir.ActivationFunctionType
    OP = mybir.AluOpType

    nc = bacc.Bacc(None, target_bir_lowering=False)
    NTT = NT * ET  # total edge tiles per core
    NE4 = NTT * 4          # (row, tile, subtile) scalar lanes
    NE32 = NTT * 32        # bessel lanes
    VEC_W = NTT * 12       # packed vec components
    EBT_W = NTT * 16       # vec part then rl part

    eb_d = nc.dram_tensor("eb", [128, EBT_W], f32, kind="ExternalInput")
    ht_d = nc.dram_tensor("ht", [NTT, 128, 512], f16, kind="ExternalInput")
    ohn_d = nc.dram_tensor("ohn", [10, NPAD], f16, kind="ExternalInput")
    consts_d = nc.dram_tensor("consts", [128, CONST_W], f32, kind="ExternalInput")
    consth_d = nc.dram_tensor("consth", [128, CONSTH_W], f16, kind="ExternalInput")
    out_d = nc.dram_tensor("out", [1, NPAD], f32, kind="ExternalOutput")

    PSUM = bass.MemorySpace.PSUM

    from contextlib import ExitStack

    with TileContext(nc) as tc:
        with ExitStack() as stack:
            cp = stack.enter_context(tc.tile_pool(name="const", bufs=1))
            htp = stack.enter_context(tc.tile_pool(name="htp", bufs=3))
            tp1 = stack.enter_context(tc.tile_pool(name="tp1", bufs=4))
            tp2 = stack.enter_context(tc.tile_pool(name="tp2", bufs=4))
            tp3 = stack.enter_context(tc.tile_pool(name="tp3", bufs=4))
            sbp = stack.enter_context(tc.tile_pool(name="sbp", bufs=3))
            sap = stack.enter_context(tc.tile_pool(name="sap", bufs=3))
            ohrp = stack.enter_context(tc.tile_pool(name="ohrp", bufs=3))
            nsb = stack.enter_context(tc.tile_pool(name="nsb", bufs=2))
            pmlp = stack.enter_context(tc.tile_pool(name="pmlp", bufs=3, space=PSUM))
            pbig = stack.enter_context(tc.tile_pool(name="pbig", bufs=3, space=PSUM))
            pmsg = stack.enter_context(tc.tile_pool(name="pmsg", bufs=2, space=PSUM))

            # ---- constants: two packed tiles, two DMAs ----
            CT = cp.tile([128, CONST_W], f32)
            nc.sync.dma_start(CT[:], consts_d[:])
            CTH = cp.tile([128, CONSTH_W], f16)
            nc.sync.dma_start(CTH[:], consth_d[:])
            OHN = cp.tile([128, NPAD], f16)
            nc.sync.dma_start(OHN[0:10, :], ohn_d[:])
            c = _spec_cols(_CONST_SPECS)
            ch = _spec_cols(_CONSTH_SPECS)
            IOTA = CT[:, c["iota"][0]:c["iota"][1]]
            CB4N = CT[:, c["cb4n"][0]:c["cb4n"][1]]
            ONEI = CT[:, c["onei"][0]:c["onei"][1]].bitcast(i32)
            NEG1I = CT[:, c["neg1i"][0]:c["neg1i"][1]].bitcast(i32)
            MAGIC = CT[:, c["magic"][0]:c["magic"][1]].bitcast(i32)
            I128 = CTH[:, ch["i128"][0]:ch["i128"][1]]
            W1A = CTH[:, ch["w1a"][0]:ch["w1a"][1]]
            W1B = CTH[:, ch["w1b"][0]:ch["w1b"][1]]
            W2 = CTH[:, ch["w2"][0]:ch["w2"][1]]
            W3 = CTH[:, ch["w3"][0]:ch["w3"][1]]
            W40A = CTH[:, ch["w40a"][0]:ch["w40a"][1]]
            W40B = CTH[:, ch["w40b"][0]:ch["w40b"][1]]
            WLIN0 = CTH[:, ch["wlin0"][0]:ch["wlin0"][1]]
            WSYM0 = CTH[0:10, ch["wsym0"][0]:ch["wsym0"][1]]
            WLIN20 = CTH[:, ch["wlin20"][0]:ch["wlin20"][1]]
            SCT = CTH[0:10, ch["sct"][0]:ch["sct"][1]]
            AE = CTH[0:10, ch["ae"][0]:ch["ae"][1]]
            WRO = CTH[:, ch["wro"][0]:ch["wro"][1]]
            OUT = cp.tile([1, NPAD], f32)

            # batched edge-chain state (computed once, up front)
            EBT = cp.tile([128, EBT_W], f32)
            nc.sync.dma_start(EBT[:], eb_d[:])
            SC = cp.tile([128, 10 * NE4], f32)
            SQ = cp.tile([128, VEC_W], f32)
            TH = cp.tile([128, 3 * NE32], f32)
            EFALL = cp.tile([128, NE32], f16)
            # rotating efT buffers; rows 32:128 stay zero (W1 pads)
            efTs = [cp.tile([128, 128], f16, name=f"efT{i}") for i in range(3)]
            for b_ in efTs:
                nc.gpsimd.memset(b_[:], 0.0)
            tc.strict_bb_all_engine_barrier()

            def sl(i):
                return SC[:, i * NE4:(i + 1) * NE4]

            ss, y, h, q, r_, x = sl(0), sl(1), sl(2), sl(3), sl(4), sl(5)
            u0, u1, u2, w = sl(6), sl(7), sl(8), sl(9)
            vec = EBT[:, 0:VEC_W]
            # ---- squared distance per (tile, subtile) ----
            nc.vector.tensor_tensor(SQ[:], vec, vec, OP.mult)
            nc.vector.tensor_reduce(
                ss, SQ[:].rearrange("p (x c) -> p x c", c=3),
                mybir.AxisListType.X, OP.add)
            nc.vector.tensor_scalar(ss, ss, EPS, None, OP.add)
            # rsqrt via bit-trick + 3 Newton steps (keeps ACT free
            # of Sqrt so the silu/sin/copy table never swaps)
            y_i = y.bitcast(i32)
            nc.vector.tensor_tensor(
                y_i, ss.bitcast(i32),
                ONEI.broadcast_to([128, NE4]), OP.arith_shift_right)
            nc.vector.tensor_tensor(
                y_i, MAGIC.broadcast_to([128, NE4]), y_i, OP.subtract)
            nc.vector.tensor_scalar(h, ss, 0.5, None, OP.mult)
            for _ in range(3):
                nc.vector.tensor_mul(q, y, y)
                nc.vector.tensor_mul(q, q, h)
                nc.vector.tensor_scalar(q, q, -1.0, 1.5, OP.mult, OP.add)
                nc.vector.tensor_mul(y, y, q)
            nc.vector.tensor_mul(r_, ss, y)   # r = ss * rsqrt(ss)
            # envelope: env = 1 + x^5*(-21 + x*(35 - 15x)), x<1
            nc.vector.tensor_scalar(x, r_, 1.0 / R_MAX, None, OP.mult)
            nc.vector.tensor_mul(u0, x, x)            # x2
            nc.vector.tensor_mul(u1, u0, u0)          # x4
            nc.vector.tensor_mul(u2, u1, x)           # x5
            nc.vector.tensor_scalar(u0, x, -15.0, 35.0, OP.mult, OP.add)  # a
            nc.vector.tensor_mul(u1, u0, x)           # b
            nc.vector.scalar_tensor_tensor(
                u0, u1, -21.0, u2, OP.add, OP.mult)   # e1 = (b-21)*x5
            nc.vector.tensor_scalar(u1, x, 1.0, None, OP.is_lt)  # mask
            nc.vector.scalar_tensor_tensor(
                u2, u0, 1.0, u1, OP.add, OP.mult)     # env
            nc.vector.scalar_tensor_tensor(
                w, u2, float(np.sqrt(2.0 / R_MAX)),
                y, OP.mult, OP.mult)                  # w = env*sqrt(2/R)*rinv

            # bessel: thn = (c_b/2pi)*r via broadcast; sin arg
            # reduced to [-0.5,0.5] then scaled by 2pi inside ACT
            thA = TH[:, 0:NE32]
            thB = TH[:, NE32:2 * NE32]
            thC = TH[:, 2 * NE32:3 * NE32]
            r_bc = r_.unsqueeze(2).broadcast_to([128, NE4, 8])
            nc.vector.tensor_tensor(
                thA.rearrange("p (x b) -> p x b", b=8),
                CB4N[:, 0:8].unsqueeze(1).broadcast_to([128, NE4, 8]),
                r_bc, OP.mult)
            nc.vector.tensor_copy(thB.bitcast(i32), thA)   # k = int(th)
            nc.vector.tensor_copy(thC, thB.bitcast(i32))   # kf = float(k)
            nc.vector.tensor_sub(thB, thA, thC)            # sa = th - kf
            nc.vector.tensor_scalar(thC, thB, 0.5, None, OP.is_gt)
            nc.vector.scalar_tensor_tensor(
                thB, thC, -1.0, thB, OP.mult, OP.add)
            nc.scalar.activation(
                thA, thB, AF.Sin, scale=float(2 * np.pi))
            w_bc = w.unsqueeze(2).broadcast_to([128, NE4, 8])
            nc.vector.tensor_tensor(
                EFALL[:].rearrange("p (x b) -> p x b", b=8),
                thA.rearrange("p (x b) -> p x b", b=8),
                w_bc, OP.mult)

            for nt in range(NT):
                msgp = pmsg.tile([128, 128], f32, tag="msgp")
                for et in range(ET):
                    ti = nt * ET + et
                    ht = htp.tile([128, 512], f16, tag="ht")
                    nc.sync.dma_start(ht[:], ht_d[ti])
                    # transpose ef slice -> efT rows 0:32 (8s..8s+8 per sub)
                    efsl = EFALL[:, ti * 32:ti * 32 + 32]
                    pefT = pmlp.tile([128, 128], f16, tag="pmlp")
                    nc.tensor.transpose(pefT[0:32, :], efsl, I128)
                    efT = efTs[ti % 3]
                    nc.scalar.copy(efT[0:32, :], pefT[0:32, :])

                    # radial MLP, 2 subtiles stacked on partitions via
                    # zero-padded block weights; full-128 contraction
                    p1 = pmlp.tile([128, 256], f32, tag="pmlp")
                    nc.tensor.matmul(p1[:, 0:128], W1A, efT[:],
                                     start=True, stop=True)
                    nc.tensor.matmul(p1[:, 128:256], W1B, efT[:],
                                     start=True, stop=True)
                    t1 = tp1.tile([128, 256], f16, tag="t1")
                    nc.scalar.activation(t1[:], p1[:], AF.Silu)
                    p2 = pmlp.tile([128, 256], f32, tag="pmlp")
                    nc.tensor.matmul(p2[:], W2, t1[:], start=True, stop=True)
                    t2 = tp2.tile([128, 256], f16, tag="t2")
                    nc.scalar.activation(t2[:], p2[:], AF.Silu)
                    p3 = pmlp.tile([128, 256], f32, tag="pmlp")
                    nc.tensor.matmul(p3[:], W3, t2[:], start=True, stop=True)
                    t3 = tp3.tile([128, 256], f16, tag="t3")
                    nc.scalar.activation(t3[:], p3[:], AF.Silu)

                    pR = pbig.tile([128, 512], f32, tag="pbig")
                    for s in range(4):
                        p_, h_ = s // 2, s % 2
                        nc.tensor.matmul(
                            pR[:, 128 * s:128 * s + 128],
                            W40A if h_ == 0 else W40B,
                            t3[:, 128 * p_:128 * p_ + 128],
                            start=True, stop=True)
                    Rs = sbp.tile([128, 512], f16, tag="Rs")
                    nc.scalar.copy(Rs[:], pR[:])

                    # 4 transposes into one psum tile; sender-channel
                    # weight applied in transposed layout via host table
                    pT = pbig.tile([128, 512], f16, tag="pbig")
                    for s in range(4):
                        nc.tensor.transpose(
                            pT[:, 128 * s:128 * s + 128],
                            Rs[:, 128 * s:128 * s + 128], I128)
                    sA = sap.tile([128, 512], f16, tag="sA")
                    nc.vector.tensor_mul(sA[:], pT[:], ht[:])
                    rlsl = EBT[:, VEC_W + ti * 4:VEC_W + ti * 4 + 4]
                    ohr = ohrp.tile([128, 512], f16, tag="ohr")
                    nc.vector.tensor_tensor(
                        ohr[:].rearrange("p (s j) -> p s j", j=128),
                        IOTA.unsqueeze(1).broadcast_to([128, 4, 128]),
                        rlsl.unsqueeze(2).broadcast_to([128, 4, 128]),
                        OP.is_equal)
                    for s in range(4):
                        nc.tensor.matmul(
                            msgp[:], sA[:, 128 * s:128 * s + 128],
                            ohr[:, 128 * s:128 * s + 128],
                            start=(et == 0 and s == 0),
                            stop=(et == ET - 1 and s == 3),
                            skip_group_check=True)

                # ---- node-tile epilogue ----
                msgs = nsb.tile([128, 128], f16, tag="msgs")
                nc.vector.tensor_scalar(
                    msgs[:], msgp[:], 1.0 / AVG, None, OP.mult)
                pf = pbig.tile([128, 128], f32, tag="pbig")
                nc.tensor.matmul(pf[:], WLIN0, msgs[:], start=True, stop=True)
                ohn_t = OHN[0:10, 128 * nt:128 * nt + 128]
                pc = pmlp.tile([128, 128], f32, tag="pmlp")
                nc.tensor.matmul(pc[:], WSYM0, ohn_t, start=True, stop=True)
                cf = nsb.tile([128, 128], f32, tag="cf")
                nc.any.tensor_copy(cf[:], pc[:])
                g = nsb.tile([128, 128], f16, tag="g")
                nc.vector.tensor_mul(g[:], cf[:], pf[:])
                po = pmlp.tile([128, 128], f32, tag="pmlp")
                nc.tensor.matmul(po[:], WLIN20, g[:], start=True, stop=False,
                                 skip_group_check=True)
                nc.tensor.matmul(po[:], SCT, ohn_t, start=False, stop=True,
                                 skip_group_check=True)
                ob = nsb.tile([128, 128], f16, tag="ob")
                nc.any.tensor_copy(ob[:], po[:])
                pe_ = pmlp.tile([128, 128], f32, tag="pmlp")
                nc.tensor.matmul(pe_[0:1, :], WRO, ob[:], start=True, stop=False,
                                 skip_group_check=True)
                nc.tensor.matmul(pe_[0:1, :], AE, ohn_t, start=False, stop=True,
                                 skip_group_check=True)
                nc.any.tensor_copy(OUT[:, 128 * nt:128 * nt + 128], pe_[0:1, :])

            nc.sync.dma_start(out_d[:], OUT[:])

    nc.compile()
    return nc


def _pack_w1(W1):
    """Pair p lhsT: rows 16p+{0:8} -> W1 cols 0:64, rows 16p+{8:16} ->
    W1 cols 64:128, zero elsewhere (full-128 contraction)."""
    out = []
    for p in range(2):
        q = np.zeros((128, 128), np.float32)
        q[16 * p + 0:16 * p + 8, 0:64] = W1
        q[16 * p + 8:16 * p + 16, 64:128] = W1
        out.append(q)
    return out


def _wbd(W):
    q = np.zeros((128, 128), np.float32)
    q[0:64, 0:64] = W
    q[64:128, 64:128] = W
    return q


def _pack_w40(W40):
    a = np.zeros((128, 128), np.float32)
    a[0:64] = W40
    b = np.zeros((128, 128), np.float32)
    b[64:128] = W40
    return a, b


def _host_prep(inputs):
    """Reduce weights to tables and build per-core edge streams."""
    pos = np.asarray(inputs["positions"], np.float32)
    shifts = np.asarray(inputs["shifts"], np.float32)
    ei = np.asarray(inputs["edge_index"])
    species = np.asarray(inputs["species"]).astype(np.int64)
    ae = np.asarray(inputs["atomic_energies"], np.float32)
    w_embed = np.asarray(inputs["w_embed"], np.float32)
    w_up = np.asarray(inputs["w_up"], np.float32)
    W1 = np.asarray(inputs["W1"], np.float32)
    W2 = np.asarray(inputs["W2"], np.float32)
    W3 = np.asarray(inputs["W3"], np.float32)
    W4 = np.asarray(inputs["W4"], np.float32)
    w_lin = np.asarray(inputs["w_lin"], np.float32)
    w_skip = np.asarray(inputs["w_skip"], np.float32)
    w_sym = np.asarray(inputs["w_sym"], np.float32)
    w_lin2 = np.asarray(inputs["w_lin2"], np.float32)
    w_readout = np.asarray(inputs["w_readout"], np.float32)

    send, recv = ei[0].astype(np.int64), ei[1].astype(np.int64)
    vec = pos[recv] - pos[send] + shifts
    rsq = (vec * vec).sum(-1)
    keep = rsq < (R_MAX * R_MAX + 1e-3)   # envelope zero beyond cutoff
    vec = vec[keep]
    sp_s = species[send[keep]]
    recv = recv[keep]

    core = recv // NPC
    loc = recv % NPC
    ntile = loc // 128
    rl = (loc % 128).astype(np.float32)

    # group edges by (core, node tile)
    order = np.lexsort((ntile, core))
    vec, sp_s, rl = vec[order], sp_s[order], rl[order]
    core, ntile = core[order], ntile[order]
    gid = core * NT + ntile
    counts = np.bincount(gid, minlength=NCORES * NT)
    SEG = int(np.ceil(counts.max() / ET_EDGES) * ET_EDGES)
    ET = SEG // ET_EDGES

    NTT = NT * ET
    EB = np.zeros((NCORES, NTT, 128, 16), np.float32)
    EB[:, :, :, 0] = 10.0  # pad edges: r=10 -> env masked to 0
    # per-edge sender table h_up[species] in transposed layout [e, k];
    # pad slots stay zero so they scatter nothing
    HU16 = (w_embed @ w_up).astype(np.float16)
    HT4 = np.zeros((NCORES, NTT, 128, 4, 128), np.float16)

    starts = np.zeros(NCORES * NT + 1, np.int64)
    np.cumsum(counts, out=starts[1:])
    for c_ in range(NCORES):
        for t in range(NT):
            g = c_ * NT + t
            a, b = starts[g], starts[g + 1]
            n = b - a
            v, s_, r_ = vec[a:b], sp_s[a:b], rl[a:b]
            idx = np.arange(n)
            tile4 = idx // 512          # which 512-edge device tile
            sub = (idx % 512) // 128    # subtile 0..3
            row = idx % 128
            ti = t * ET + tile4
            EB[c_, ti, row, 3 * sub + 0] = v[:, 0]
            EB[c_, ti, row, 3 * sub + 1] = v[:, 1]
            EB[c_, ti, row, 3 * sub + 2] = v[:, 2]
            EB[c_, ti, row, 12 + sub] = r_
            HT4[c_, ti, row, sub] = HU16[s_]

    OHN = np.zeros((NCORES, 10, NPAD), np.float16)
    for c_ in range(NCORES):
        sp_c = species[c_ * NPC:(c_ + 1) * NPC]
        OHN[c_, sp_c, np.arange(NPC)] = 1.0

    n_ = np.arange(1, NB + 1, dtype=np.float32)
    cb4n_row = np.tile((n_ * np.pi / R_MAX / (2 * np.pi)).astype(np.float32), 4)
    w1a, w1b = _pack_w1(W1)
    w40a, w40b = _pack_w40(np.ascontiguousarray(W4.reshape(64, K, 4)[:, :, 0]))
    mats = {
        "iota": np.broadcast_to(
            np.arange(128, dtype=np.float32), (128, 128)).copy(),
        "cb4n": np.broadcast_to(cb4n_row, (128, 32)).copy(),
        "onei": np.full((128, 1), 1, np.int32).view(np.float32),
        "neg1i": np.full((128, 1), -1, np.int32).view(np.float32),
        "magic": np.full((128, 1), 0x5F3759DF, np.int32).view(np.float32),
    }
    cols = _spec_cols(_CONST_SPECS)
    packed = np.zeros((128, CONST_W), np.float32)
    for k_, m in mats.items():
        a, b = cols[k_]
        packed[:m.shape[0], a:b] = m

    math = {
        "i128": np.eye(128, dtype=np.float32),
        "w1a": w1a, "w1b": w1b,
        "w2": _wbd(W2), "w3": _wbd(W3),
        "w40a": w40a, "w40b": w40b,
        "wlin0": np.ascontiguousarray(w_lin[0]),
        "wsym0": np.ascontiguousarray(w_sym[0]),
        "wlin20": np.ascontiguousarray(w_lin2[0]),
        "sct": np.ascontiguousarray(
            np.einsum("zk,zkj->zj", w_embed, w_skip) / np.sqrt(Z)),
        "ae": ae.reshape(10, 1).copy(),
        "wro": w_readout.reshape(128, 1).copy(),
    }
    colsh = _spec_cols(_CONSTH_SPECS)
    packedh = np.zeros((128, CONSTH_W), np.float16)
    for k_, m in math.items():
        a, b = colsh[k_]
        packedh[:m.shape[0], a:b] = m.astype(np.float16)

    # device layouts: EBT = [128, vec (NTT*12) | rl (NTT*4)] per core;
    # HT = [NTT, 128, 512] per core (one DMA per edge tile)
    EBT = np.concatenate([
        EB[:, :, :, 0:12].transpose(0, 2, 1, 3).reshape(NCORES, 128, NTT * 12),
        EB[:, :, :, 12:16].transpose(0, 2, 1, 3).reshape(NCORES, 128, NTT * 4),
    ], axis=2)
    HT = HT4.reshape(NCORES, NTT, 128, 512)
    return ET, EBT, HT, OHN, {"consts": packed, "consth": packedh}


def kernel(**inputs):
    global LAST_RESULTS
    from concourse.bass_utils import run_bass_kernel_spmd

    ET, EBT, HT, OHN, consts = _host_prep(inputs)
    if ET not in _prog_cache:
        _prog_cache[ET] = _build_program(ET)
    nc = _prog_cache[ET]

    in_maps = []
    for c_ in range(NCORES):
        m = dict(consts)
        m["eb"] = np.ascontiguousarray(EBT[c_])
        m["ht"] = np.ascontiguousarray(HT[c_])
        m["ohn"] = np.ascontiguousarray(OHN[c_])
        in_maps.append(m)

    res = run_bass_kernel_spmd(
        nc, in_maps, core_ids=list(range(NCORES)), trace=TRACE)
    LAST_RESULTS = res

    out = np.concatenate(
        [res.results[c_]["out"][0, :NPC] for c_ in range(NCORES)])
    return out.astype(np.float32)



# revision 42
# speedup vs baseline: 1.6339x; 1.6339x over previous
"""MACE-style GNN message passing on 8 Trainium2 NeuronCores.

Only the l=0 (scalar) channel of the reference network reaches the output:
Y[:,0] == 1 and the readout consumes out[:, :, 0] alone, so the whole l>=1
spherical-harmonic pipeline is dead code.  What remains per edge is the
radial MLP (8->64->64->64->128), a per-sender-species channel scale, and a
scatter-add over receivers.  Node-side work collapses to per-species table
lookups (implemented as one-hot matmuls) plus three K x K matmuls.

Sharding: receivers are range-partitioned over the 8 cores (1000 nodes
each, padded to 8 tiles of 128).  Each core gets the edges targeting its
nodes, grouped by 128-node tile and padded to a uniform segment length so
all cores run one SPMD program.  Edges with r >= r_max (zero envelope) are
dropped on the host.

All matmuls keep operands on full 128-partition tiles at base partition 0
(tile_position (0,0)); sub-128 contractions are realized with zero-padded
block weights.  Partial-partition operands on compute-written tiles
misread on this hardware.
"""

import sys
import numpy as np

sys.path.insert(0, "/opt/trn_rl_repo")

R_MAX = 5.0
EPS = 1e-9
AVG = 16.0
N_NODES = 8000
Z = 10
K = 128
NB = 8
NCORES = 8
NPC = N_NODES // NCORES       # nodes per core
NT = 8                        # node tiles per core (128 nodes each)
NPAD = NT * 128               # padded nodes per core
ET_EDGES = 512                # edges per device tile (4 x 128 subtiles)

_CONST_SPECS = [
    ("iota", 128), ("cb4n", 32),
    ("onei", 1), ("neg1i", 1), ("magic", 1),
]
CONST_W = sum(w for _, w in _CONST_SPECS)

# fp16 constants (matmul weights; PE runs fp16 at 1 cycle/row vs 4 for fp32)
_CONSTH_SPECS = [
    ("i128", 128),
    ("w1a", 128), ("w1b", 128),
    ("w2", 128), ("w3", 128),
    ("w4eo", 256),
    ("wlin0", 128), ("wsym0", 128),
    ("wlin20", 128), ("sct", 128), ("ae", 1), ("wro", 1),
]
CONSTH_W = sum(w for _, w in _CONSTH_SPECS)


def _spec_cols(specs):
    cols, off = {}, 0
    for name, w in specs:
        cols[name] = (off, off + w)
        off += w
    return cols


TRACE = False
LAST_RESULTS = None

_prog_cache = {}


def _build_program(ET):
    """Build the SPMD Bass program for ET 512-edge tiles per node tile."""
    from concourse import bass, bacc, mybir
    from concourse.tile import TileContext

    f32 = mybir.dt.float32
    f16 = mybir.dt.float16
    i32 = mybir.dt.int32
    AF = mybir.ActivationFunctionType
    OP = mybir.AluOpType

    nc = bacc.Bacc(None, target_bir_lowering=False)
    NTT = NT * ET  # total edge tiles per core
    NE4 = NTT * 4          # (row, tile, subtile) scalar lanes
    NE32 = NTT * 32        # bessel lanes
    VEC_W = NTT * 12       # packed vec components
    EBT_W = NTT * 16       # vec part then rl part

    eb_d = nc.dram_tensor("eb", [128, EBT_W], f32, kind="ExternalInput")
    ht_d = nc.dram_tensor("ht", [NTT, 128, 512], f16, kind="ExternalInput")
    ohn_d = nc.dram_tensor("ohn", [10, NPAD], f16, kind="ExternalInput")
    consts_d = nc.dram_tensor("consts", [128, CONST_W], f32, kind="ExternalInput")
    consth_d = nc.dram_tensor("consth", [128, CONSTH_W], f16, kind="ExternalInput")
    out_d = nc.dram_tensor("out", [1, NPAD], f32, kind="ExternalOutput")

    PSUM = bass.MemorySpace.PSUM

    from contextlib import ExitStack

    with TileContext(nc) as tc:
        with ExitStack() as stack:
            cp = stack.enter_context(tc.tile_pool(name="const", bufs=1))
            htp = stack.enter_context(tc.tile_pool(name="htp", bufs=3))
            tp1 = stack.enter_context(tc.tile_pool(name="tp1", bufs=4))
            tp2 = stack.enter_context(tc.tile_pool(name="tp2", bufs=4))
            tp3 = stack.enter_context(tc.tile_pool(name="tp3", bufs=4))
            sap = stack.enter_context(tc.tile_pool(name="sap", bufs=3))
            ohrp = stack.enter_context(tc.tile_pool(name="ohrp", bufs=3))
            nsb = stack.enter_context(tc.tile_pool(name="nsb", bufs=2))
            pmlp = stack.enter_context(tc.tile_pool(name="pmlp", bufs=3, space=PSUM))
            pbig = stack.enter_context(tc.tile_pool(name="pbig", bufs=2, space=PSUM))
            pmsg = stack.enter_context(tc.tile_pool(name="pmsg", bufs=2, space=PSUM))

            # ---- constants: two packed tiles, two DMAs ----
            CT = cp.tile([128, CONST_W], f32)
            nc.sync.dma_start(CT[:], consts_d[:])
            CTH = cp.tile([128, CONSTH_W], f16)
            nc.sync.dma_start(CTH[:], consth_d[:])
            OHN = cp.tile([128, NPAD], f16)
            nc.sync.dma_start(OHN[0:10, :], ohn_d[:])
            c = _spec_cols(_CONST_SPECS)
            ch = _spec_cols(_CONSTH_SPECS)
            IOTA = CT[:, c["iota"][0]:c["iota"][1]]
            CB4N = CT[:, c["cb4n"][0]:c["cb4n"][1]]
            ONEI = CT[:, c["onei"][0]:c["onei"][1]].bitcast(i32)
            NEG1I = CT[:, c["neg1i"][0]:c["neg1i"][1]].bitcast(i32)
            MAGIC = CT[:, c["magic"][0]:c["magic"][1]].bitcast(i32)
            I128 = CTH[:, ch["i128"][0]:ch["i128"][1]]
            W1A = CTH[:, ch["w1a"][0]:ch["w1a"][1]]
            W1B = CTH[:, ch["w1b"][0]:ch["w1b"][1]]
            W2 = CTH[:, ch["w2"][0]:ch["w2"][1]]
            W3 = CTH[:, ch["w3"][0]:ch["w3"][1]]
            W4EO = CTH[:, ch["w4eo"][0]:ch["w4eo"][1]]
            WLIN0 = CTH[:, ch["wlin0"][0]:ch["wlin0"][1]]
            WSYM0 = CTH[0:10, ch["wsym0"][0]:ch["wsym0"][1]]
            WLIN20 = CTH[:, ch["wlin20"][0]:ch["wlin20"][1]]
            SCT = CTH[0:10, ch["sct"][0]:ch["sct"][1]]
            AE = CTH[0:10, ch["ae"][0]:ch["ae"][1]]
            WRO = CTH[:, ch["wro"][0]:ch["wro"][1]]
            OUT = cp.tile([1, NPAD], f32)
            MS = cp.tile([128, NPAD], f16)   # all node-tile messages

            # batched edge-chain state (computed once, up front)
            EBT = cp.tile([128, EBT_W], f32)
            nc.sync.dma_start(EBT[:], eb_d[:])
            SC = cp.tile([128, 10 * NE4], f32)
            SQ = cp.tile([128, VEC_W], f32)
            TH = cp.tile([128, 3 * NE32], f32)
            EFALL = cp.tile([128, NE32], f16)
            # rotating efT buffers; rows 32:128 stay zero (W1 pads)
            efTs = [cp.tile([128, 128], f16, name=f"efT{i}") for i in range(3)]
            for b_ in efTs:
                nc.gpsimd.memset(b_[:], 0.0)
            tc.strict_bb_all_engine_barrier()

            def sl(i):
                return SC[:, i * NE4:(i + 1) * NE4]

            ss, y, h, q, r_, x = sl(0), sl(1), sl(2), sl(3), sl(4), sl(5)
            u0, u1, u2, w = sl(6), sl(7), sl(8), sl(9)
            vec = EBT[:, 0:VEC_W]
            # ---- squared distance per (tile, subtile) ----
            nc.vector.tensor_tensor(SQ[:], vec, vec, OP.mult)
            nc.vector.tensor_reduce(
                ss, SQ[:].rearrange("p (x c) -> p x c", c=3),
                mybir.AxisListType.X, OP.add)
            nc.vector.tensor_scalar(ss, ss, EPS, None, OP.add)
            # rsqrt via bit-trick + 3 Newton steps (keeps ACT free
            # of Sqrt so the silu/sin/copy table never swaps)
            y_i = y.bitcast(i32)
            nc.vector.tensor_tensor(
                y_i, ss.bitcast(i32),
                ONEI.broadcast_to([128, NE4]), OP.arith_shift_right)
            nc.vector.tensor_tensor(
                y_i, MAGIC.broadcast_to([128, NE4]), y_i, OP.subtract)
            nc.vector.tensor_scalar(h, ss, 0.5, None, OP.mult)
            for _ in range(3):
                nc.vector.tensor_mul(q, y, y)
                nc.vector.tensor_mul(q, q, h)
                nc.vector.tensor_scalar(q, q, -1.0, 1.5, OP.mult, OP.add)
                nc.vector.tensor_mul(y, y, q)
            nc.vector.tensor_mul(r_, ss, y)   # r = ss * rsqrt(ss)
            # envelope: env = 1 + x^5*(-21 + x*(35 - 15x)), x<1
            nc.vector.tensor_scalar(x, r_, 1.0 / R_MAX, None, OP.mult)
            nc.vector.tensor_mul(u0, x, x)            # x2
            nc.vector.tensor_mul(u1, u0, u0)          # x4
            nc.vector.tensor_mul(u2, u1, x)           # x5
            nc.vector.tensor_scalar(u0, x, -15.0, 35.0, OP.mult, OP.add)  # a
            nc.vector.tensor_mul(u1, u0, x)           # b
            nc.vector.scalar_tensor_tensor(
                u0, u1, -21.0, u2, OP.add, OP.mult)   # e1 = (b-21)*x5
            nc.vector.tensor_scalar(u1, x, 1.0, None, OP.is_lt)  # mask
            nc.vector.scalar_tensor_tensor(
                u2, u0, 1.0, u1, OP.add, OP.mult)     # env
            nc.vector.scalar_tensor_tensor(
                w, u2, float(np.sqrt(2.0 / R_MAX)),
                y, OP.mult, OP.mult)                  # w = env*sqrt(2/R)*rinv

            # bessel: thn = (c_b/2pi)*r via broadcast; sin arg
            # reduced to [-0.5,0.5] then scaled by 2pi inside ACT
            thA = TH[:, 0:NE32]
            thB = TH[:, NE32:2 * NE32]
            thC = TH[:, 2 * NE32:3 * NE32]
            r_bc = r_.unsqueeze(2).broadcast_to([128, NE4, 8])
            nc.vector.tensor_tensor(
                thA.rearrange("p (x b) -> p x b", b=8),
                CB4N[:, 0:8].unsqueeze(1).broadcast_to([128, NE4, 8]),
                r_bc, OP.mult)
            nc.vector.tensor_copy(thB.bitcast(i32), thA)   # k = int(th)
            nc.vector.tensor_copy(thC, thB.bitcast(i32))   # kf = float(k)
            nc.vector.tensor_sub(thB, thA, thC)            # sa = th - kf
            nc.vector.tensor_scalar(thC, thB, 0.5, None, OP.is_gt)
            nc.vector.scalar_tensor_tensor(
                thB, thC, -1.0, thB, OP.mult, OP.add)
            nc.scalar.activation(
                thA, thB, AF.Sin, scale=float(2 * np.pi))
            w_bc = w.unsqueeze(2).broadcast_to([128, NE4, 8])
            nc.vector.tensor_tensor(
                EFALL[:].rearrange("p (x b) -> p x b", b=8),
                thA.rearrange("p (x b) -> p x b", b=8),
                w_bc, OP.mult)

            for nt in range(NT):
                msgp = pmsg.tile([128, 128], f32, tag="msgp")
                for et in range(ET):
                    ti = nt * ET + et
                    ht = htp.tile([128, 512], f16, tag="ht")
                    nc.sync.dma_start(ht[:], ht_d[ti])
                    # transpose ef slice -> efT rows 0:32 (8s..8s+8 per sub)
                    efsl = EFALL[:, ti * 32:ti * 32 + 32]
                    pefT = pmlp.tile([128, 128], f16, tag="pmlp")
                    nc.tensor.transpose(pefT[0:32, :], efsl, I128)
                    efT = efTs[ti % 3]
                    nc.scalar.copy(efT[0:32, :], pefT[0:32, :])

                    # radial MLP, 2 subtiles stacked on partitions via
                    # zero-padded block weights; full-128 contraction
                    p1 = pmlp.tile([128, 256], f32, tag="pmlp")
                    nc.tensor.matmul(p1[:, 0:128], W1A, efT[:],
                                     start=True, stop=True)
                    nc.tensor.matmul(p1[:, 128:256], W1B, efT[:],
                                     start=True, stop=True)
                    t1 = tp1.tile([128, 256], f16, tag="t1")
                    nc.scalar.activation(t1[:], p1[:], AF.Silu)
                    p2 = pmlp.tile([128, 256], f32, tag="pmlp")
                    nc.tensor.matmul(p2[:], W2, t1[:], start=True, stop=True)
                    t2 = tp2.tile([128, 256], f16, tag="t2")
                    nc.scalar.activation(t2[:], p2[:], AF.Silu)
                    p3 = pmlp.tile([128, 256], f32, tag="pmlp")
                    nc.tensor.matmul(p3[:], W3, t2[:], start=True, stop=True)
                    t3 = tp3.tile([128, 256], f16, tag="t3")
                    nc.scalar.activation(t3[:], p3[:], AF.Silu)

                    # W4 with swapped operands: out = t3_block^T @ W4EO
                    # lands [edges, K] directly (no transposes, no copy);
                    # W4EO rows 0:64 -> even-sub cols, 64:128 -> odd-sub
                    pRT = pbig.tile([128, 512], f32, tag="pbig")
                    nc.tensor.matmul(pRT[:, 0:256], t3[:, 0:128], W4EO,
                                     start=True, stop=True)
                    nc.tensor.matmul(pRT[:, 256:512], t3[:, 128:256], W4EO,
                                     start=True, stop=True)
                    sA = sap.tile([128, 512], f16, tag="sA")
                    nc.vector.tensor_mul(sA[:], pRT[:], ht[:])
                    rlsl = EBT[:, VEC_W + ti * 4:VEC_W + ti * 4 + 4]
                    ohr = ohrp.tile([128, 512], f16, tag="ohr")
                    nc.vector.tensor_tensor(
                        ohr[:].rearrange("p (s j) -> p s j", j=128),
                        IOTA.unsqueeze(1).broadcast_to([128, 4, 128]),
                        rlsl.unsqueeze(2).broadcast_to([128, 4, 128]),
                        OP.is_equal)
                    for s in range(4):
                        nc.tensor.matmul(
                            msgp[:], sA[:, 128 * s:128 * s + 128],
                            ohr[:, 128 * s:128 * s + 128],
                            start=(et == 0 and s == 0),
                            stop=(et == ET - 1 and s == 3),
                            skip_group_check=True)

                # node-tile messages into the batched epilogue buffer
                nc.vector.tensor_scalar(
                    MS[:, 128 * nt:128 * nt + 128], msgp[:],
                    1.0 / AVG, None, OP.mult)

            # ---- batched epilogue over all node tiles, 512-col blocks ----
            for hb in range(NPAD // 512):
                sl0, sl1 = hb * 512, hb * 512 + 512
                pf = pbig.tile([128, 512], f32, tag="pbig")
                nc.tensor.matmul(pf[:], WLIN0, MS[:, sl0:sl1],
                                 start=True, stop=True)
                ohn_t = OHN[0:10, sl0:sl1]
                pc = pbig.tile([128, 512], f32, tag="pbig")
                nc.tensor.matmul(pc[:], WSYM0, ohn_t, start=True, stop=True)
                cf = nsb.tile([128, 512], f16, tag="cf")
                nc.any.tensor_copy(cf[:], pc[:])
                g = nsb.tile([128, 512], f16, tag="g")
                nc.vector.tensor_mul(g[:], pf[:], cf[:])
                po = pbig.tile([128, 512], f32, tag="pbig")
                nc.tensor.matmul(po[:], WLIN20, g[:], start=True, stop=False,
                                 skip_group_check=True)
                nc.tensor.matmul(po[:], SCT, ohn_t, start=False, stop=True,
                                 skip_group_check=True)
                ob = nsb.tile([128, 512], f16, tag="ob")
                nc.any.tensor_copy(ob[:], po[:])
                pe_ = pmlp.tile([128, 512], f32, tag="pe", bufs=1)
                nc.tensor.matmul(pe_[0:1, :], WRO, ob[:], start=True, stop=False,
                                 skip_group_check=True)
                nc.tensor.matmul(pe_[0:1, :], AE, ohn_t, start=False, stop=True,
                                 skip_group_check=True)
                nc.any.tensor_copy(OUT[:, sl0:sl1], pe_[0:1, :])

            nc.sync.dma_start(out_d[:], OUT[:])

    nc.compile()
    return nc


def _pack_w1(W1):
    """Pair p lhsT: rows 16p+{0:8} -> W1 cols 0:64, rows 16p+{8:16} ->
    W1 cols 64:128, zero elsewhere (full-128 contraction)."""
    out = []
    for p in range(2):
        q = np.zeros((128, 128), np.float32)
        q[16 * p + 0:16 * p + 8, 0:64] = W1
        q[16 * p + 8:16 * p + 16, 64:128] = W1
        out.append(q)
    return out


def _wbd(W):
    q = np.zeros((128, 128), np.float32)
    q[0:64, 0:64] = W
    q[64:128, 64:128] = W
    return q


def _pack_w4eo(W40):
    """[128, 256]: rows 0:64 -> cols 0:128 (even sub), rows 64:128 ->
    cols 128:256 (odd sub)."""
    q = np.zeros((128, 256), np.float32)
    q[0:64, 0:128] = W40
    q[64:128, 128:256] = W40
    return q


def _host_prep(inputs):
    """Reduce weights to tables and build per-core edge streams."""
    pos = np.asarray(inputs["positions"], np.float32)
    shifts = np.asarray(inputs["shifts"], np.float32)
    ei = np.asarray(inputs["edge_index"])
    species = np.asarray(inputs["species"]).astype(np.int64)
    ae = np.asarray(inputs["atomic_energies"], np.float32)
    w_embed = np.asarray(inputs["w_embed"], np.float32)
    w_up = np.asarray(inputs["w_up"], np.float32)
    W1 = np.asarray(inputs["W1"], np.float32)
    W2 = np.asarray(inputs["W2"], np.float32)
    W3 = np.asarray(inputs["W3"], np.float32)
    W4 = np.asarray(inputs["W4"], np.float32)
    w_lin = np.asarray(inputs["w_lin"], np.float32)
    w_skip = np.asarray(inputs["w_skip"], np.float32)
    w_sym = np.asarray(inputs["w_sym"], np.float32)
    w_lin2 = np.asarray(inputs["w_lin2"], np.float32)
    w_readout = np.asarray(inputs["w_readout"], np.float32)

    send, recv = ei[0].astype(np.int64), ei[1].astype(np.int64)
    vec = pos[recv] - pos[send] + shifts
    rsq = (vec * vec).sum(-1)
    keep = rsq < (R_MAX * R_MAX + 1e-3)   # envelope zero beyond cutoff
    vec = vec[keep]
    sp_s = species[send[keep]]
    recv = recv[keep]

    core = recv // NPC
    loc = recv % NPC
    ntile = loc // 128
    rl = (loc % 128).astype(np.float32)

    # group edges by (core, node tile)
    order = np.lexsort((ntile, core))
    vec, sp_s, rl = vec[order], sp_s[order], rl[order]
    core, ntile = core[order], ntile[order]
    gid = core * NT + ntile
    counts = np.bincount(gid, minlength=NCORES * NT)
    SEG = int(np.ceil(counts.max() / ET_EDGES) * ET_EDGES)
    ET = SEG // ET_EDGES

    NTT = NT * ET
    EB = np.zeros((NCORES, NTT, 128, 16), np.float32)
    EB[:, :, :, 0] = 10.0  # pad edges: r=10 -> env masked to 0
    # per-edge sender table h_up[species] in transposed layout [e, k];
    # pad slots stay zero so they scatter nothing
    HU16 = (w_embed @ w_up).astype(np.float16)
    HT4 = np.zeros((NCORES, NTT, 128, 4, 128), np.float16)

    starts = np.zeros(NCORES * NT + 1, np.int64)
    np.cumsum(counts, out=starts[1:])
    for c_ in range(NCORES):
        for t in range(NT):
            g = c_ * NT + t
            a, b = starts[g], starts[g + 1]
            n = b - a
            v, s_, r_ = vec[a:b], sp_s[a:b], rl[a:b]
            idx = np.arange(n)
            tile4 = idx // 512          # which 512-edge device tile
            sub = (idx % 512) // 128    # subtile 0..3
            row = idx % 128
            ti = t * ET + tile4
            EB[c_, ti, row, 3 * sub + 0] = v[:, 0]
            EB[c_, ti, row, 3 * sub + 1] = v[:, 1]
            EB[c_, ti, row, 3 * sub + 2] = v[:, 2]
            EB[c_, ti, row, 12 + sub] = r_
            HT4[c_, ti, row, sub] = HU16[s_]

    OHN = np.zeros((NCORES, 10, NPAD), np.float16)
    for c_ in range(NCORES):
        sp_c = species[c_ * NPC:(c_ + 1) * NPC]
        OHN[c_, sp_c, np.arange(NPC)] = 1.0

    n_ = np.arange(1, NB + 1, dtype=np.float32)
    cb4n_row = np.tile((n_ * np.pi / R_MAX / (2 * np.pi)).astype(np.float32), 4)
    w1a, w1b = _pack_w1(W1)
    w4eo = _pack_w4eo(np.ascontiguousarray(W4.reshape(64, K, 4)[:, :, 0]))
    mats = {
        "iota": np.broadcast_to(
            np.arange(128, dtype=np.float32), (128, 128)).copy(),
        "cb4n": np.broadcast_to(cb4n_row, (128, 32)).copy(),
        "onei": np.full((128, 1), 1, np.int32).view(np.float32),
        "neg1i": np.full((128, 1), -1, np.int32).view(np.float32),
        "magic": np.full((128, 1), 0x5F3759DF, np.int32).view(np.float32),
    }
    cols = _spec_cols(_CONST_SPECS)
    packed = np.zeros((128, CONST_W), np.float32)
    for k_, m in mats.items():
        a, b = cols[k_]
        packed[:m.shape[0], a:b] = m

    math = {
        "i128": np.eye(128, dtype=np.float32),
        "w1a": w1a, "w1b": w1b,
        "w2": _wbd(W2), "w3": _wbd(W3),
        "w4eo": w4eo,
        "wlin0": np.ascontiguousarray(w_lin[0]),
        "wsym0": np.ascontiguousarray(w_sym[0]),
        "wlin20": np.ascontiguousarray(w_lin2[0]),
        "sct": np.ascontiguousarray(
            np.einsum("zk,zkj->zj", w_embed, w_skip) / np.sqrt(Z)),
        "ae": ae.reshape(10, 1).copy(),
        "wro": w_readout.reshape(128, 1).copy(),
    }
    colsh = _spec_cols(_CONSTH_SPECS)
    packedh = np.zeros((128, CONSTH_W), np.float16)
    for k_, m in math.items():
        a, b = colsh[k_]
        packedh[:m.shape[0], a:b] = m.astype(np.float16)

    # device layouts: EBT = [128, vec (NTT*12) | rl (NTT*4)] per core;
    # HT = [NTT, 128, 512] per core (one DMA per edge tile)
    EBT = np.concatenate([
        EB[:, :, :, 0:12].transpose(0, 2, 1, 3).reshape(NCORES, 128, NTT * 12),
        EB[:, :, :, 12:16].transpose(0, 2, 1, 3).reshape(NCORES, 128, NTT * 4),
    ], axis=2)
    HT = HT4.reshape(NCORES, NTT, 128, 512)
    return ET, EBT, HT, OHN, {"consts": packed, "consth": packedh}


def kernel(**inputs):
    global LAST_RESULTS
    from concourse.bass_utils import run_bass_kernel_spmd

    ET, EBT, HT, OHN, consts = _host_prep(inputs)
    if ET not in _prog_cache:
        _prog_cache[ET] = _build_program(ET)
    nc = _prog_cache[ET]

    in_maps = []
    for c_ in range(NCORES):
        m = dict(consts)
        m["eb"] = np.ascontiguousarray(EBT[c_])
        m["ht"] = np.ascontiguousarray(HT[c_])
        m["ohn"] = np.ascontiguousarray(OHN[c_])
        in_maps.append(m)

    res = run_bass_kernel_spmd(
        nc, in_maps, core_ids=list(range(NCORES)), trace=TRACE)
    LAST_RESULTS = res

    out = np.concatenate(
        [res.results[c_]["out"][0, :NPC] for c_ in range(NCORES)])
    return out.astype(np.float32)



# revision 43
# speedup vs baseline: 1.6722x; 1.0235x over previous
"""MACE-style GNN message passing on 8 Trainium2 NeuronCores.

Only the l=0 (scalar) channel of the reference network reaches the output:
Y[:,0] == 1 and the readout consumes out[:, :, 0] alone, so the whole l>=1
spherical-harmonic pipeline is dead code.  What remains per edge is the
radial MLP (8->64->64->64->128), a per-sender-species channel scale, and a
scatter-add over receivers.  Node-side work collapses to per-species table
lookups (implemented as one-hot matmuls) plus three K x K matmuls.

Sharding: receivers are range-partitioned over the 8 cores (1000 nodes
each, padded to 8 tiles of 128).  Each core gets the edges targeting its
nodes, grouped by 128-node tile and padded to a uniform segment length so
all cores run one SPMD program.  Edges with r >= r_max (zero envelope) are
dropped on the host.

All matmuls keep operands on full 128-partition tiles at base partition 0
(tile_position (0,0)); sub-128 contractions are realized with zero-padded
block weights.  Partial-partition operands on compute-written tiles
misread on this hardware.
"""

import sys
import numpy as np

sys.path.insert(0, "/opt/trn_rl_repo")

R_MAX = 5.0
EPS = 1e-9
AVG = 16.0
N_NODES = 8000
Z = 10
K = 128
NB = 8
NCORES = 8
NPC = N_NODES // NCORES       # nodes per core
NT = 8                        # node tiles per core (128 nodes each)
NPAD = NT * 128               # padded nodes per core
ET_EDGES = 512                # edges per device tile (4 x 128 subtiles)

_CONST_SPECS = [
    ("iota", 128), ("cb4n", 32),
    ("onei", 1), ("neg1i", 1), ("magic", 1),
]
CONST_W = sum(w for _, w in _CONST_SPECS)

# fp16 constants (matmul weights; PE runs fp16 at 1 cycle/row vs 4 for fp32)
_CONSTH_SPECS = [
    ("i128", 128),
    ("w1a", 128), ("w1b", 128),
    ("w2", 128), ("w3", 128),
    ("w4eo", 256),
    ("wlin0", 128), ("wsym0", 128),
    ("wlin20", 128), ("sct", 128), ("ae", 1), ("wro", 1),
]
CONSTH_W = sum(w for _, w in _CONSTH_SPECS)


def _spec_cols(specs):
    cols, off = {}, 0
    for name, w in specs:
        cols[name] = (off, off + w)
        off += w
    return cols


TRACE = False
LAST_RESULTS = None

_prog_cache = {}


def _build_program(ET):
    """Build the SPMD Bass program for ET 512-edge tiles per node tile."""
    from concourse import bass, bacc, mybir
    from concourse.tile import TileContext

    f32 = mybir.dt.float32
    f16 = mybir.dt.float16
    i32 = mybir.dt.int32
    AF = mybir.ActivationFunctionType
    OP = mybir.AluOpType

    nc = bacc.Bacc(None, target_bir_lowering=False)
    NTT = NT * ET  # total edge tiles per core
    NE4 = NTT * 4          # (row, tile, subtile) scalar lanes
    NE32 = NTT * 32        # bessel lanes
    VEC_W = NTT * 12       # packed vec components
    EBT_W = NTT * 16       # vec part then rl part

    eb_d = nc.dram_tensor("eb", [128, EBT_W], f32, kind="ExternalInput")
    ht_d = nc.dram_tensor("ht", [NTT, 128, 512], f16, kind="ExternalInput")
    ohn_d = nc.dram_tensor("ohn", [10, NPAD], f16, kind="ExternalInput")
    consts_d = nc.dram_tensor("consts", [128, CONST_W], f32, kind="ExternalInput")
    consth_d = nc.dram_tensor("consth", [128, CONSTH_W], f16, kind="ExternalInput")
    out_d = nc.dram_tensor("out", [1, NPAD], f32, kind="ExternalOutput")

    PSUM = bass.MemorySpace.PSUM

    from contextlib import ExitStack

    with TileContext(nc) as tc:
        with ExitStack() as stack:
            cp = stack.enter_context(tc.tile_pool(name="const", bufs=1))
            htp = stack.enter_context(tc.tile_pool(name="htp", bufs=3))
            tp1 = stack.enter_context(tc.tile_pool(name="tp1", bufs=4))
            tp2 = stack.enter_context(tc.tile_pool(name="tp2", bufs=4))
            tp3 = stack.enter_context(tc.tile_pool(name="tp3", bufs=4))
            sap = stack.enter_context(tc.tile_pool(name="sap", bufs=3))
            ohrp = stack.enter_context(tc.tile_pool(name="ohrp", bufs=3))
            nsb = stack.enter_context(tc.tile_pool(name="nsb", bufs=2))
            pmlp = stack.enter_context(tc.tile_pool(name="pmlp", bufs=3, space=PSUM))
            pbig = stack.enter_context(tc.tile_pool(name="pbig", bufs=2, space=PSUM))
            pmsg = stack.enter_context(tc.tile_pool(name="pmsg", bufs=2, space=PSUM))

            # ---- constants: two packed tiles, two DMAs ----
            CT = cp.tile([128, CONST_W], f32)
            nc.sync.dma_start(CT[:], consts_d[:])
            CTH = cp.tile([128, CONSTH_W], f16)
            nc.sync.dma_start(CTH[:], consth_d[:])
            OHN = cp.tile([128, NPAD], f16)
            nc.sync.dma_start(OHN[0:10, :], ohn_d[:])
            c = _spec_cols(_CONST_SPECS)
            ch = _spec_cols(_CONSTH_SPECS)
            IOTA = CT[:, c["iota"][0]:c["iota"][1]]
            CB4N = CT[:, c["cb4n"][0]:c["cb4n"][1]]
            ONEI = CT[:, c["onei"][0]:c["onei"][1]].bitcast(i32)
            NEG1I = CT[:, c["neg1i"][0]:c["neg1i"][1]].bitcast(i32)
            MAGIC = CT[:, c["magic"][0]:c["magic"][1]].bitcast(i32)
            I128 = CTH[:, ch["i128"][0]:ch["i128"][1]]
            W1A = CTH[:, ch["w1a"][0]:ch["w1a"][1]]
            W1B = CTH[:, ch["w1b"][0]:ch["w1b"][1]]
            W2 = CTH[:, ch["w2"][0]:ch["w2"][1]]
            W3 = CTH[:, ch["w3"][0]:ch["w3"][1]]
            W4EO = CTH[:, ch["w4eo"][0]:ch["w4eo"][1]]
            WLIN0 = CTH[:, ch["wlin0"][0]:ch["wlin0"][1]]
            WSYM0 = CTH[0:10, ch["wsym0"][0]:ch["wsym0"][1]]
            WLIN20 = CTH[:, ch["wlin20"][0]:ch["wlin20"][1]]
            SCT = CTH[0:10, ch["sct"][0]:ch["sct"][1]]
            AE = CTH[0:10, ch["ae"][0]:ch["ae"][1]]
            WRO = CTH[:, ch["wro"][0]:ch["wro"][1]]
            OUT = cp.tile([1, NPAD], f32)
            MS = cp.tile([128, NPAD], f16)   # all node-tile messages

            # batched edge-chain state (computed once, up front)
            EBT = cp.tile([128, EBT_W], f32)
            nc.sync.dma_start(EBT[:], eb_d[:])
            SC = cp.tile([128, 10 * NE4], f32)
            SQ = cp.tile([128, VEC_W], f32)
            TH = cp.tile([128, 3 * NE32], f32)
            EFALL = cp.tile([128, NE32], f16)
            # rotating efT buffers; rows 32:128 stay zero (W1 pads)
            efTs = [cp.tile([128, 128], f16, name=f"efT{i}") for i in range(3)]
            for b_ in efTs:
                nc.gpsimd.memset(b_[:], 0.0)
            tc.strict_bb_all_engine_barrier()

            # chain emitted in node-tile-pair groups, interleaved one
            # pair ahead of the matmul loop so DVE work overlaps PE
            NPAIR = NT // 2
            NE4G = NE4 // NPAIR
            NE32G = NE32 // NPAIR
            VWG = VEC_W // NPAIR

            def emit_chain(g):
                def slg(i):
                    a = i * NE4 + g * NE4G
                    return SC[:, a:a + NE4G]

                ss, y, h, q = slg(0), slg(1), slg(2), slg(3)
                r_, x = slg(4), slg(5)
                u0, u1, u2, w = slg(6), slg(7), slg(8), slg(9)
                vec = EBT[:, g * VWG:(g + 1) * VWG]
                sq = SQ[:, g * VWG:(g + 1) * VWG]
                thA = TH[:, g * NE32G:(g + 1) * NE32G]
                thB = TH[:, NE32 + g * NE32G:NE32 + (g + 1) * NE32G]
                thC = TH[:, 2 * NE32 + g * NE32G:2 * NE32 + (g + 1) * NE32G]
                efg = EFALL[:, g * NE32G:(g + 1) * NE32G]
                # ---- squared distance per (tile, subtile) ----
                nc.vector.tensor_tensor(sq, vec, vec, OP.mult)
                nc.vector.tensor_reduce(
                    ss, sq.rearrange("p (x c) -> p x c", c=3),
                    mybir.AxisListType.X, OP.add)
                nc.vector.tensor_scalar(ss, ss, EPS, None, OP.add)
                # rsqrt via bit-trick + 3 Newton steps (keeps ACT free
                # of Sqrt so the silu/sin/copy table never swaps)
                y_i = y.bitcast(i32)
                nc.vector.tensor_tensor(
                    y_i, ss.bitcast(i32),
                    ONEI.broadcast_to([128, NE4G]), OP.arith_shift_right)
                nc.vector.tensor_tensor(
                    y_i, MAGIC.broadcast_to([128, NE4G]), y_i, OP.subtract)
                nc.vector.tensor_scalar(h, ss, 0.5, None, OP.mult)
                for _ in range(3):
                    nc.vector.tensor_mul(q, y, y)
                    nc.vector.tensor_mul(q, q, h)
                    nc.vector.tensor_scalar(q, q, -1.0, 1.5, OP.mult, OP.add)
                    nc.vector.tensor_mul(y, y, q)
                nc.vector.tensor_mul(r_, ss, y)   # r = ss * rsqrt(ss)
                # envelope: env = 1 + x^5*(-21 + x*(35 - 15x)), x<1
                nc.vector.tensor_scalar(x, r_, 1.0 / R_MAX, None, OP.mult)
                nc.vector.tensor_mul(u0, x, x)            # x2
                nc.vector.tensor_mul(u1, u0, u0)          # x4
                nc.vector.tensor_mul(u2, u1, x)           # x5
                nc.vector.tensor_scalar(
                    u0, x, -15.0, 35.0, OP.mult, OP.add)  # a
                nc.vector.tensor_mul(u1, u0, x)           # b
                nc.vector.scalar_tensor_tensor(
                    u0, u1, -21.0, u2, OP.add, OP.mult)   # e1 = (b-21)*x5
                nc.vector.tensor_scalar(u1, x, 1.0, None, OP.is_lt)  # mask
                nc.vector.scalar_tensor_tensor(
                    u2, u0, 1.0, u1, OP.add, OP.mult)     # env
                nc.vector.scalar_tensor_tensor(
                    w, u2, float(np.sqrt(2.0 / R_MAX)),
                    y, OP.mult, OP.mult)                  # w = env*sqrt(2/R)*rinv

                # bessel: thn = (c_b/2pi)*r via broadcast; sin arg
                # reduced to [-0.5,0.5] then scaled by 2pi inside ACT
                r_bc = r_.unsqueeze(2).broadcast_to([128, NE4G, 8])
                nc.vector.tensor_tensor(
                    thA.rearrange("p (x b) -> p x b", b=8),
                    CB4N[:, 0:8].unsqueeze(1).broadcast_to([128, NE4G, 8]),
                    r_bc, OP.mult)
                nc.vector.tensor_copy(thB.bitcast(i32), thA)   # k = int(th)
                nc.vector.tensor_copy(thC, thB.bitcast(i32))   # kf = float(k)
                nc.vector.tensor_sub(thB, thA, thC)            # sa = th - kf
                nc.vector.tensor_scalar(thC, thB, 0.5, None, OP.is_gt)
                nc.vector.scalar_tensor_tensor(
                    thB, thC, -1.0, thB, OP.mult, OP.add)
                nc.scalar.activation(
                    thA, thB, AF.Sin, scale=float(2 * np.pi))
                w_bc = w.unsqueeze(2).broadcast_to([128, NE4G, 8])
                nc.vector.tensor_tensor(
                    efg.rearrange("p (x b) -> p x b", b=8),
                    thA.rearrange("p (x b) -> p x b", b=8),
                    w_bc, OP.mult)

            emit_chain(0)
            for nt in range(NT):
                if nt % 2 == 0 and nt // 2 + 1 < NPAIR:
                    emit_chain(nt // 2 + 1)
                msgp = pmsg.tile([128, 128], f32, tag="msgp")
                for et in range(ET):
                    ti = nt * ET + et
                    ht = htp.tile([128, 512], f16, tag="ht")
                    nc.sync.dma_start(ht[:], ht_d[ti])
                    # transpose ef slice -> efT rows 0:32 (8s..8s+8 per sub)
                    efsl = EFALL[:, ti * 32:ti * 32 + 32]
                    pefT = pmlp.tile([128, 128], f16, tag="pmlp")
                    nc.tensor.transpose(pefT[0:32, :], efsl, I128)
                    efT = efTs[ti % 3]
                    nc.scalar.copy(efT[0:32, :], pefT[0:32, :])

                    # radial MLP, 2 subtiles stacked on partitions via
                    # zero-padded block weights; full-128 contraction
                    p1 = pmlp.tile([128, 256], f32, tag="pmlp")
                    nc.tensor.matmul(p1[:, 0:128], W1A, efT[:],
                                     start=True, stop=True)
                    nc.tensor.matmul(p1[:, 128:256], W1B, efT[:],
                                     start=True, stop=True)
                    t1 = tp1.tile([128, 256], f16, tag="t1")
                    nc.scalar.activation(t1[:], p1[:], AF.Silu)
                    p2 = pmlp.tile([128, 256], f32, tag="pmlp")
                    nc.tensor.matmul(p2[:], W2, t1[:], start=True, stop=True)
                    t2 = tp2.tile([128, 256], f16, tag="t2")
                    nc.scalar.activation(t2[:], p2[:], AF.Silu)
                    p3 = pmlp.tile([128, 256], f32, tag="pmlp")
                    nc.tensor.matmul(p3[:], W3, t2[:], start=True, stop=True)
                    t3 = tp3.tile([128, 256], f16, tag="t3")
                    nc.scalar.activation(t3[:], p3[:], AF.Silu)

                    # W4 with swapped operands: out = t3_block^T @ W4EO
                    # lands [edges, K] directly (no transposes, no copy);
                    # W4EO rows 0:64 -> even-sub cols, 64:128 -> odd-sub
                    pRT = pbig.tile([128, 512], f32, tag="pbig")
                    nc.tensor.matmul(pRT[:, 0:256], t3[:, 0:128], W4EO,
                                     start=True, stop=True)
                    nc.tensor.matmul(pRT[:, 256:512], t3[:, 128:256], W4EO,
                                     start=True, stop=True)
                    sA = sap.tile([128, 512], f16, tag="sA")
                    nc.vector.tensor_mul(sA[:], pRT[:], ht[:])
                    rlsl = EBT[:, VEC_W + ti * 4:VEC_W + ti * 4 + 4]
                    ohr = ohrp.tile([128, 512], f16, tag="ohr")
                    nc.vector.tensor_tensor(
                        ohr[:].rearrange("p (s j) -> p s j", j=128),
                        IOTA.unsqueeze(1).broadcast_to([128, 4, 128]),
                        rlsl.unsqueeze(2).broadcast_to([128, 4, 128]),
                        OP.is_equal)
                    for s in range(4):
                        nc.tensor.matmul(
                            msgp[:], sA[:, 128 * s:128 * s + 128],
                            ohr[:, 128 * s:128 * s + 128],
                            start=(et == 0 and s == 0),
                            stop=(et == ET - 1 and s == 3),
                            skip_group_check=True)

                # node-tile messages into the batched epilogue buffer
                nc.vector.tensor_scalar(
                    MS[:, 128 * nt:128 * nt + 128], msgp[:],
                    1.0 / AVG, None, OP.mult)

            # ---- batched epilogue over all node tiles, 512-col blocks ----
            for hb in range(NPAD // 512):
                sl0, sl1 = hb * 512, hb * 512 + 512
                pf = pbig.tile([128, 512], f32, tag="pbig")
                nc.tensor.matmul(pf[:], WLIN0, MS[:, sl0:sl1],
                                 start=True, stop=True)
                ohn_t = OHN[0:10, sl0:sl1]
                pc = pbig.tile([128, 512], f32, tag="pbig")
                nc.tensor.matmul(pc[:], WSYM0, ohn_t, start=True, stop=True)
                cf = nsb.tile([128, 512], f16, tag="cf")
                nc.any.tensor_copy(cf[:], pc[:])
                g = nsb.tile([128, 512], f16, tag="g")
                nc.vector.tensor_mul(g[:], pf[:], cf[:])
                po = pbig.tile([128, 512], f32, tag="pbig")
                nc.tensor.matmul(po[:], WLIN20, g[:], start=True, stop=False,
                                 skip_group_check=True)
                nc.tensor.matmul(po[:], SCT, ohn_t, start=False, stop=True,
                                 skip_group_check=True)
                ob = nsb.tile([128, 512], f16, tag="ob")
                nc.any.tensor_copy(ob[:], po[:])
                pe_ = pmlp.tile([128, 512], f32, tag="pe", bufs=1)
                nc.tensor.matmul(pe_[0:1, :], WRO, ob[:], start=True, stop=False,
                                 skip_group_check=True)
                nc.tensor.matmul(pe_[0:1, :], AE, ohn_t, start=False, stop=True,
                                 skip_group_check=True)
                nc.any.tensor_copy(OUT[:, sl0:sl1], pe_[0:1, :])

            nc.sync.dma_start(out_d[:], OUT[:])

    nc.compile()
    return nc


def _pack_w1(W1):
    """Pair p lhsT: rows 16p+{0:8} -> W1 cols 0:64, rows 16p+{8:16} ->
    W1 cols 64:128, zero elsewhere (full-128 contraction)."""
    out = []
    for p in range(2):
        q = np.zeros((128, 128), np.float32)
        q[16 * p + 0:16 * p + 8, 0:64] = W1
        q[16 * p + 8:16 * p + 16, 64:128] = W1
        out.append(q)
    return out


def _wbd(W):
    q = np.zeros((128, 128), np.float32)
    q[0:64, 0:64] = W
    q[64:128, 64:128] = W
    return q


def _pack_w4eo(W40):
    """[128, 256]: rows 0:64 -> cols 0:128 (even sub), rows 64:128 ->
    cols 128:256 (odd sub)."""
    q = np.zeros((128, 256), np.float32)
    q[0:64, 0:128] = W40
    q[64:128, 128:256] = W40
    return q


def _host_prep(inputs):
    """Reduce weights to tables and build per-core edge streams."""
    pos = np.asarray(inputs["positions"], np.float32)
    shifts = np.asarray(inputs["shifts"], np.float32)
    ei = np.asarray(inputs["edge_index"])
    species = np.asarray(inputs["species"]).astype(np.int64)
    ae = np.asarray(inputs["atomic_energies"], np.float32)
    w_embed = np.asarray(inputs["w_embed"], np.float32)
    w_up = np.asarray(inputs["w_up"], np.float32)
    W1 = np.asarray(inputs["W1"], np.float32)
    W2 = np.asarray(inputs["W2"], np.float32)
    W3 = np.asarray(inputs["W3"], np.float32)
    W4 = np.asarray(inputs["W4"], np.float32)
    w_lin = np.asarray(inputs["w_lin"], np.float32)
    w_skip = np.asarray(inputs["w_skip"], np.float32)
    w_sym = np.asarray(inputs["w_sym"], np.float32)
    w_lin2 = np.asarray(inputs["w_lin2"], np.float32)
    w_readout = np.asarray(inputs["w_readout"], np.float32)

    send, recv = ei[0].astype(np.int64), ei[1].astype(np.int64)
    vec = pos[recv] - pos[send] + shifts
    rsq = (vec * vec).sum(-1)
    keep = rsq < (R_MAX * R_MAX + 1e-3)   # envelope zero beyond cutoff
    vec = vec[keep]
    sp_s = species[send[keep]]
    recv = recv[keep]

    core = recv // NPC
    loc = recv % NPC
    ntile = loc // 128
    rl = (loc % 128).astype(np.float32)

    # group edges by (core, node tile)
    order = np.lexsort((ntile, core))
    vec, sp_s, rl = vec[order], sp_s[order], rl[order]
    core, ntile = core[order], ntile[order]
    gid = core * NT + ntile
    counts = np.bincount(gid, minlength=NCORES * NT)
    SEG = int(np.ceil(counts.max() / ET_EDGES) * ET_EDGES)
    ET = SEG // ET_EDGES

    NTT = NT * ET
    EB = np.zeros((NCORES, NTT, 128, 16), np.float32)
    EB[:, :, :, 0] = 10.0  # pad edges: r=10 -> env masked to 0
    # per-edge sender table h_up[species] in transposed layout [e, k];
    # pad slots stay zero so they scatter nothing
    HU16 = (w_embed @ w_up).astype(np.float16)
    HT4 = np.zeros((NCORES, NTT, 128, 4, 128), np.float16)

    starts = np.zeros(NCORES * NT + 1, np.int64)
    np.cumsum(counts, out=starts[1:])
    for c_ in range(NCORES):
        for t in range(NT):
            g = c_ * NT + t
            a, b = starts[g], starts[g + 1]
            n = b - a
            v, s_, r_ = vec[a:b], sp_s[a:b], rl[a:b]
            idx = np.arange(n)
            tile4 = idx // 512          # which 512-edge device tile
            sub = (idx % 512) // 128    # subtile 0..3
            row = idx % 128
            ti = t * ET + tile4
            EB[c_, ti, row, 3 * sub + 0] = v[:, 0]
            EB[c_, ti, row, 3 * sub + 1] = v[:, 1]
            EB[c_, ti, row, 3 * sub + 2] = v[:, 2]
            EB[c_, ti, row, 12 + sub] = r_
            HT4[c_, ti, row, sub] = HU16[s_]

    OHN = np.zeros((NCORES, 10, NPAD), np.float16)
    for c_ in range(NCORES):
        sp_c = species[c_ * NPC:(c_ + 1) * NPC]
        OHN[c_, sp_c, np.arange(NPC)] = 1.0

    n_ = np.arange(1, NB + 1, dtype=np.float32)
    cb4n_row = np.tile((n_ * np.pi / R_MAX / (2 * np.pi)).astype(np.float32), 4)
    w1a, w1b = _pack_w1(W1)
    w4eo = _pack_w4eo(np.ascontiguousarray(W4.reshape(64, K, 4)[:, :, 0]))
    mats = {
        "iota": np.broadcast_to(
            np.arange(128, dtype=np.float32), (128, 128)).copy(),
        "cb4n": np.broadcast_to(cb4n_row, (128, 32)).copy(),
        "onei": np.full((128, 1), 1, np.int32).view(np.float32),
        "neg1i": np.full((128, 1), -1, np.int32).view(np.float32),
        "magic": np.full((128, 1), 0x5F3759DF, np.int32).view(np.float32),
    }
    cols = _spec_cols(_CONST_SPECS)
    packed = np.zeros((128, CONST_W), np.float32)
    for k_, m in mats.items():
        a, b = cols[k_]
        packed[:m.shape[0], a:b] = m

    math = {
        "i128": np.eye(128, dtype=np.float32),
        "w1a": w1a, "w1b": w1b,
        "w2": _wbd(W2), "w3": _wbd(W3),
        "w4eo": w4eo,
        "wlin0": np.ascontiguousarray(w_lin[0]),
        "wsym0": np.ascontiguousarray(w_sym[0]),
        "wlin20": np.ascontiguousarray(w_lin2[0]),
        "sct": np.ascontiguousarray(
            np.einsum("zk,zkj->zj", w_embed, w_skip) / np.sqrt(Z)),
        "ae": ae.reshape(10, 1).copy(),
        "wro": w_readout.reshape(128, 1).copy(),
    }
    colsh = _spec_cols(_CONSTH_SPECS)
    packedh = np.zeros((128, CONSTH_W), np.float16)
    for k_, m in math.items():
        a, b = colsh[k_]
        packedh[:m.shape[0], a:b] = m.astype(np.float16)

    # device layouts: EBT = [128, vec (NTT*12) | rl (NTT*4)] per core;
    # HT = [NTT, 128, 512] per core (one DMA per edge tile)
    EBT = np.concatenate([
        EB[:, :, :, 0:12].transpose(0, 2, 1, 3).reshape(NCORES, 128, NTT * 12),
        EB[:, :, :, 12:16].transpose(0, 2, 1, 3).reshape(NCORES, 128, NTT * 4),
    ], axis=2)
    HT = HT4.reshape(NCORES, NTT, 128, 512)
    return ET, EBT, HT, OHN, {"consts": packed, "consth": packedh}


def kernel(**inputs):
    global LAST_RESULTS
    from concourse.bass_utils import run_bass_kernel_spmd

    ET, EBT, HT, OHN, consts = _host_prep(inputs)
    if ET not in _prog_cache:
        _prog_cache[ET] = _build_program(ET)
    nc = _prog_cache[ET]

    in_maps = []
    for c_ in range(NCORES):
        m = dict(consts)
        m["eb"] = np.ascontiguousarray(EBT[c_])
        m["ht"] = np.ascontiguousarray(HT[c_])
        m["ohn"] = np.ascontiguousarray(OHN[c_])
        in_maps.append(m)

    res = run_bass_kernel_spmd(
        nc, in_maps, core_ids=list(range(NCORES)), trace=TRACE)
    LAST_RESULTS = res

    out = np.concatenate(
        [res.results[c_]["out"][0, :NPC] for c_ in range(NCORES)])
    return out.astype(np.float32)

